# revision 76
# baseline (speedup 1.0000x reference)
"""DGCNN forward kernel for Trainium2 (Bass/Tile), data-parallel over batch.

Each of the 8 NeuronCores processes one point cloud (B=8).

Per conv block (the key rewrite): with w = [w_nb | w_ctr] ([O, 2C]),
    y[o,n,k] = (w_nb @ X)[o, idx[n,k]] + ((w_ctr - w_nb) @ X)[o, n]
and since the BN scale g/sqrt(v+eps) > 0 and LeakyReLU is monotone,
    max_k lrelu(bn(y)) = lrelu(bn(max_k A[o, idx[n,k]] + D[o, n])).
So each block is: knn (PE matmul + DVE top-20) -> row gather of A^T from a
DRAM table (gpsimd dma_gather) -> max-tree -> +D via PSUM
transpose-accumulate (PE) -> fused BN+LeakyReLU (ACT Prelu).

knn top-20: the pd PSUM->SBUF copy quantizes to int16 at per-block scale S
(ACT, free), one DVE add packs the column index into the f32 fraction
(p = t16 + m/2048, exact since |S*pd|<=8191 -> 13+11 bits), then top-8 of
each 128-column chunk (8 short max8s) and top-20 of the 64 candidates
(max8/match_replace x2 on [128,64]). Indices are recovered exactly as
(p*2048) & 2047, clamped to [0,1023].

v2 restructure (cost-model driven):
- pd/nxx matmuls in float32r (1 cyc/row at >=256-wide out vs 4 for f32;
  knn selection only -- quantized to 13 bits anyway).
- Per-block phases: A (pd+topk for all 8 tiles) -> batched index
  recover + fold (f16 sel/idx, one pass per block instead of per tile)
  -> C (gather into 8 dedicated g slots, max-tree, +D, Prelu).
- b0/b1 max-trees and the embedding max-reduce run on the Pool engine
  (idle between gathers); b2/b3 trees stay on DVE (bf16 2x).
- weT/wf0T weight loads moved off the ACT queue (SP, deferred past
  block 0) so block 0 is not blocked behind 19us of DMA.
"""

import numpy as np
import ml_dtypes

_BF16 = ml_dtypes.bfloat16

import concourse.bacc as bacc
import concourse.bass as bass
import concourse.mybir as mybir
from concourse.bass_utils import run_bass_kernel_spmd
from concourse.masks import make_identity
from concourse.tile import TileContext, add_dep_helper

F32 = mybir.dt.float32
F32R = mybir.dt.float32r
U32 = mybir.dt.uint32
I16 = mybir.dt.int16
I32 = mybir.dt.int32
F16 = mybir.dt.float16
BF16 = mybir.dt.bfloat16
AF = mybir.ActivationFunctionType
ALU = mybir.AluOpType
AX = mybir.AxisListType

N = 1024
P = 128
T = 8  # row tiles per cloud
K = 20
EPS = 1e-5
ALPHA = 0.2
NEG = -3.0e38
BLOCKS = [(3, 64), (64, 64), (64, 128), (128, 256)]  # (C_in, C_out)
# per-block quantization scale: t' = f16(1536 + SW*pd) rounds to an exact
# integer in [1024,2048) (f16 quantum there is 1.0), a 10-bit quantization
# of pd (range measured on real+syn inputs, ~6% margin to the 511 cap)
S_PD = [19.0, 30.2, 36.2, 17.8]
ts = bass.ts


def _bn_sb(bnt):
    """bnt: [128, nch, 2] sbuf (cols: scale, bias; host-precomputed)."""
    return bnt[:, :, 0], bnt[:, :, 1]


class _Rot:
    """Slot-reuse WAR tracker: a new writer of a rotating buffer slot must
    wait for the previous round's last cross-engine reader."""

    def __init__(self, bufs):
        self.bufs, self.i, self.readers = bufs, 0, {}
        self.cur = 0

    def write(self, winst):
        self.cur = self.i % self.bufs
        self.i += 1
        r = self.readers.get(self.cur)
        if r is not None:
            add_dep_helper(winst.ins, r.ins, reason="slot-war")

    def read(self, rinst):
        self.readers[self.cur] = rinst


def build(debug=False):
    nc = bacc.Bacc()

    x = nc.dram_tensor("x", [3, N], F32, kind="ExternalInput")
    wta, wtb, bnt_d = [], [], []
    for i, (c, o) in enumerate(BLOCKS):
        wta.append(nc.dram_tensor(f"wta{i}", [c, o], F32, kind="ExternalInput"))
        wtb.append(nc.dram_tensor(f"wtb{i}", [c, o], F32, kind="ExternalInput"))
        bnt_d.append(nc.dram_tensor(f"bnt{i}", [o, 2], F32, kind="ExternalInput"))
    weT = nc.dram_tensor("weT", [512, 1024], BF16, kind="ExternalInput")
    bneT = nc.dram_tensor("bneT", [1024, 2], F32, kind="ExternalInput")
    wf0T = nc.dram_tensor("wf0T", [2048, 512], F32, kind="ExternalInput")
    bnf0T = nc.dram_tensor("bnf0T", [512, 2], F32, kind="ExternalInput")
    wf1T = nc.dram_tensor("wf1T", [512, 256], F32, kind="ExternalInput")
    bnf1T = nc.dram_tensor("bnf1T", [256, 2], F32, kind="ExternalInput")
    wfinT = nc.dram_tensor("wfinT", [256, 64], F32, kind="ExternalInput")
    bfin = nc.dram_tensor("bfin", [64, 1], F32, kind="ExternalInput")
    sel = nc.dram_tensor("sel", [8, 128, 128], F16, kind="ExternalInput")
    ones_d = nc.dram_tensor("onesd", [1, 128], F32, kind="ExternalInput")
    mones_d = nc.dram_tensor("monesd", [128, 1], F32, kind="ExternalInput")
    negxx0_d = nc.dram_tensor("negxx0d", [1, N], F32, kind="ExternalInput")
    iota_d = nc.dram_tensor("iotaf", [128, N], F16, kind="ExternalInput")
    out_d = nc.dram_tensor("out", [64, 1], F32, kind="ExternalOutput")
    dbg = {}
    if debug:
        for nm, shp in [("x1d", [64, N]), ("x2d", [64, N]), ("x3d", [128, N]),
                        ("hd", [128, 16])]:
            dbg[nm] = nc.dram_tensor(nm, shp, F32, kind="ExternalOutput")

    with TileContext(nc) as tc:
        with (
            tc.tile_pool(name="const", bufs=1) as cpool,
            tc.tile_pool(name="wpool", bufs=1) as wpool,
            tc.tile_pool(name="xpool", bufs=1) as xpool,
            tc.tile_pool(name="dram", bufs=2, space="DRAM") as dpool,
        ):
            # ---- x + block-0 criticals first: unblocks block-0 prep ----
            x0_sb = xpool.tile([3, N], F32R)
            nc.sync.dma_start(out=x0_sb[:], in_=x[:].bitcast(F32R))
            wa0_sb = wpool.tile([3, 128], F32, tag="wta0")
            nc.vector.memset(wa0_sb[:], 0.0)
            nc.sync.dma_start(out=wa0_sb[:, :64], in_=wta[0][:])
            wb0_sb = wpool.tile([3, 64], F32, tag="wtb0")
            nc.sync.dma_start(out=wb0_sb[:], in_=wtb[0][:])
            ones = cpool.tile([1, 128], F32R)
            nc.sync.dma_start(out=ones[:], in_=ones_d[:].bitcast(F32R))
            mones = cpool.tile([128, 1], F32R)
            nc.sync.dma_start(out=mones[:], in_=mones_d[:].bitcast(F32R))
            bt0_sb = wpool.tile([128, 1, 2], F32, tag="bnt0")
            nc.vector.memset(bt0_sb[:], 0.0)
            nc.sync.dma_start(out=bt0_sb[:64, 0, :], in_=bnt_d[0][:])
            # ---- constants ----
            negxx0 = cpool.tile([1, N], F32R)
            nc.sync.dma_start(out=negxx0[:], in_=negxx0_d[:].bitcast(F32R))
            iota_sb = cpool.tile([128, N], F16)
            nc.sync.dma_start(out=iota_sb[:], in_=iota_d[:])
            sel_sb = cpool.tile([128, 8, 128], F16)
            nc.sync.dma_start(out=sel_sb[:], in_=sel[:].rearrange("g p r -> p g r"))
            ident = cpool.tile([128, 128], F32)
            make_identity(nc, ident[:])
            ident16 = cpool.tile([128, 128], F16)
            nc.scalar.copy(out=ident16[:], in_=ident[:])
            ones16 = cpool.tile([1, 128], F16)
            nc.vector.memset(ones16[:], 1.0)

            # ---- small weights (weT/wf0T deferred to after block 0) ----
            wta_sb, wtb_sb = [wa0_sb], [wb0_sb]
            for i, (c, o) in enumerate(BLOCKS):
                if i == 0:
                    continue
                wa = wpool.tile([c, 128 if o < 128 else o], F32R if i == 3 else F32, tag=f"wta{i}")
                if o < 128:
                    nc.vector.memset(wa[:], 0.0)
                nc.sync.dma_start(out=wa[:, :o], in_=wta[i][:].bitcast(F32R) if i == 3 else wta[i][:])
                wb = wpool.tile([c, o], F32, tag=f"wtb{i}")
                nc.sync.dma_start(out=wb[:], in_=wtb[i][:])
                wta_sb.append(wa)
                wtb_sb.append(wb)
            wf1T_sb = wpool.tile([128, 4, 256], F32)
            wfinT_sb = wpool.tile([128, 2, 64], F32)
            bfin_sb = wpool.tile([64, 1], F32)
            bnt_sb = [bt0_sb]
            for i, (c, o) in enumerate(BLOCKS):
                if i == 0:
                    continue
                nch = (o + 127) // 128
                bt = wpool.tile([128, nch, 2], F32, tag=f"bnt{i}")
                src = bnt_d[i][:]
                if nch == 1:
                    nc.vector.memset(bt[:], 0.0)
                    nc.sync.dma_start(out=bt[:o, 0, :], in_=src)
                else:
                    nc.sync.dma_start(out=bt[:], in_=src.rearrange("(c p) q -> p c q", p=128))
                bnt_sb.append(bt)
            bneT_sb = wpool.tile([128, 8, 2], F32)
            bnf0T_sb = wpool.tile([128, 4, 2], F32)
            bnf1T_sb = wpool.tile([128, 2, 2], F32)
            weT_bf = wpool.tile([128, 4, 1024], BF16)
            wf0T_sb = wpool.tile([128, 16, 512], F32)

            # ---- X tiles (block inputs/outputs) ----
            x1_sb = xpool.tile([64, N], F32R)
            x2_sb = xpool.tile([64, N], F32R)
            x12_sb = xpool.tile([128, N], F32)
            x3_sb = xpool.tile([128, N], F32R)
            x12_bf = xpool.tile([128, N], BF16)
            x3_bf = xpool.tile([128, N], BF16)
            x4_bf = xpool.tile([128, 2, N], BF16)

            with (
                tc.tile_pool(name="work", bufs=2) as wk,
                tc.tile_pool(name="wk1", bufs=1) as wk1,
                tc.tile_pool(name="gp", bufs=1) as gp,
                tc.tile_pool(name="pdp", bufs=2) as pdp,
                tc.tile_pool(name="psA", bufs=1, space="PSUM") as psA,
                tc.tile_pool(name="ps1", bufs=1, space="PSUM") as ps1,
            ):
                R = {k: _Rot(b_) for k, b_ in (
                    [("pd0", 1), ("pd1", 1), ("at", 1), ("fold", 1), ("xn", 1),
                     ("t16", 2), ("ppk0", 1), ("ppk1", 1), ("idx16a", 2), ("eb", 2),
                     ("cand", 2), ("idxf", 2),
                     ("p_sb", 2), ("mf", 2), ("sq", 1),
                     ("negxx", 1), ("at_sb", 2), ("at_dram", 2)]
                    + [(f"g{t}", 1) for t in range(4)])}

                # per-block constants up-front (input-only, off the critical path)
                wd_all, scbi_all = [], []
                for b, (C, O) in enumerate(BLOCKS):
                    wd = wk1.tile([128, 256], F32, tag=f"wd{b}")
                    nc.vector.tensor_tensor(out=wd[:C, :O], in0=wtb_sb[b][:], in1=wta_sb[b][:, :O], op=ALU.subtract)
                    wd_all.append(wd)
                    scbi_all.append(_bn_sb(bnt_sb[b]))

                for b, (C, O) in enumerate(BLOCKS):
                    X = [x0_sb, x1_sb, x2_sb, x3_sb][b]
                    OC = (O + 127) // 128  # output chunks
                    w_nbT = wta_sb[b][:, :max(O, 128)]
                    wd = wd_all[b]
                    sc, bi = scbi_all[b]

                    # X tiles are stored f32r (the Prelu writes f32r, which
                    # the BIR verifier requires for f32r-matmul inputs); f32
                    # consumers (the xn D-matmul) view them as plain f32 bits
                    def XS(sl):
                        return X[:C, sl].bitcast(F32) if b > 0 else X[:, sl].bitcast(F32)

                    def XR(sl):
                        return X[:C, sl] if b > 0 else X[:, sl]

                    # -xx/2 (the 2x scale on X^T X is dropped: top-k is
                    # invariant under positive scaling of the row); this chain
                    # gates every pd matmul of the block, so prioritize it.
                    # Block 0's row is a pure function of the input x and is
                    # precomputed on the host (skips the startup sq chain).
                    if b == 0:
                        negxx = negxx0
                    else:
                        sq = wk1.tile([128, N], F32R, tag="sq")
                        negxx = wk1.tile([1, N], F32R, tag="negxx")
                        with tc.high_priority():
                            R["sq"].write(nc.scalar.activation(out=sq[:C, 0:512], in_=XS(slice(0, 512)), func=AF.Square))
                            nc.scalar.activation(out=sq[:C, 512:1024], in_=XS(slice(512, 1024)), func=AF.Square)
                            for h in range(2):
                                nxx_ps = psA.tile([1, 512], F32, tag=f"pd{h}")
                                mmn = nc.tensor.matmul(out=nxx_ps[:], lhsT=mones[:C, :],
                                                       rhs=sq[:C, ts(h, 512)], start=True, stop=True)
                                R[f"pd{h}"].write(mmn)
                                R["sq"].read(mmn)
                                cpn = nc.scalar.copy(out=negxx[:, ts(h, 512)], in_=nxx_ps[:])
                                R[f"pd{h}"].read(cpn)
                                if h == 0:
                                    R["negxx"].write(cpn)

                    # A^T table -> DRAM, always bf16 viewed as u32 pairs
                    # (halves the modeled per-row element count). Rows must be
                    # 256B multiples for the gather, so O=64 tables are padded
                    # to 128 bf16 cols; the pad is never read downstream.
                    OP = max(O, 128)  # padded row width in bf16 elems
                    GW = OP // 2  # gathered row width (u32 elems)
                    at_dram = dpool.tile([N, GW], U32, tag="at")

                    def emit_at_table():
                        for t in range(T):
                            at_ps = ps1.tile([128, 256], F32, tag="at")
                            if O >= 256:
                                mm = nc.tensor.matmul(out=at_ps[:, :OP], lhsT=XR(ts(t, 128)),
                                                      rhs=w_nbT, start=True, stop=True)
                            else:
                                # w_nbT is zero-padded to 128 cols for O=64 so
                                # the whole 256B table row is initialized
                                mm = nc.tensor.matmul(out=at_ps[:, :OP],
                                                      lhsT=XS(ts(t, 128)),
                                                      rhs=w_nbT,
                                                      start=True, stop=True)
                            R["at"].write(mm)
                            at_sb = wk.tile([128, 256], BF16, tag="at_sb")
                            cp = nc.scalar.copy(out=at_sb[:, :OP], in_=at_ps[:, :OP])
                            R["at"].read(cp)
                            R["at_sb"].write(cp)
                            if t == 0:
                                R["at_dram"].write(cp)  # proxy: table write begins
                            dmai = nc.sync.dma_start(out=at_dram[ts(t, 128), :],
                                                     in_=at_sb[:, :OP].bitcast(U32))
                            R["at_sb"].read(dmai)

                    NT = 4  # tiles per round (A of round r+1 overlaps C of r)

                    def stage_a(t, tl, p24a):
                        # pairwise-distance tile (row-constant dropped):
                        # pd[n, m] = x_n.x_m - |x_m|^2/2, quantized by the ACT
                        # copy to an exact f16 integer in [1024,2048)
                        tf = pdp.tile([128, N], F16, tag="t16")
                        ppk0 = psA.tile([128, 512], F32, tag="ppk0")
                        ppk1 = psA.tile([128, 512], F32, tag="ppk1")
                        ppk = [ppk0, ppk1]
                        for h in range(2):
                            pd_ps = psA.tile([128, 512], F32, tag=f"pd{h}")
                            mm1 = nc.tensor.matmul(out=pd_ps[:],
                                                   lhsT=XR(ts(t, 128)),
                                                   rhs=XR(ts(h, 512)),
                                                   start=True, stop=False)
                            R[f"pd{h}"].write(mm1)
                            mm2 = nc.tensor.matmul(out=pd_ps[:], lhsT=ones[:],
                                                   rhs=negxx[:, ts(h, 512)], start=False, stop=True,
                                                   skip_group_check=True)
                            R["negxx"].read(mm2)
                            cp = nc.scalar.activation(out=tf[:, ts(h, 512)], in_=pd_ps[:],
                                                      func=AF.Copy, scale=S_PD[b], bias=1536.0)
                            R[f"pd{h}"].read(cp)
                            if h == 0:
                                R["t16"].write(cp)

                        # pack the column index into the f32 fraction on the PE:
                        # p = t' + m/2048 via identity-copy + broadcast-add into
                        # PSUM (frees ~36us of DVE TT adds). One bank per half
                        # so tile t+1's pack overlaps tile t's top-8 scans.
                        for h in range(2):
                            mp1 = nc.tensor.matmul(out=ppk[h][:], lhsT=ident16[:],
                                                   rhs=tf[:, ts(h, 512)], start=True, stop=False)
                            R[f"ppk{h}"].write(mp1)
                            mp2 = nc.tensor.matmul(out=ppk[h][:], lhsT=ones16[:],
                                                   rhs=iota_sb[0:1, ts(h, 512)], start=False, stop=True,
                                                   skip_group_check=True)
                            if h == 1:
                                R["t16"].read(mp2)

                        # top-8 of each 256-chunk, then top-20 of the 32
                        # candidates (the packed fraction is the global column
                        # index, so chunk width only affects the scan split)
                        cand = wk.tile([128, 4, 8], F32, tag="cand")
                        for c4 in range(4):
                            pc = ppk[c4 // 2][:].rearrange("p (c m) -> p c m", c=2)
                            mx = nc.vector.max(out=cand[:, c4, :], in_=pc[:, c4 % 2, :])
                            if c4 == 0:
                                R["cand"].write(mx)
                            if c4 == 1:
                                R["ppk0"].read(mx)
                            if c4 == 3:
                                R["ppk1"].read(mx)
                        cand_f = cand[:].rearrange("p c m -> p (c m)")
                        nc.vector.max(out=p24a[:, tl, 0:8], in_=cand_f)
                        nc.vector.match_replace(out=cand_f, in_to_replace=p24a[:, tl, 0:8],
                                                in_values=cand_f, imm_value=NEG)
                        nc.vector.max(out=p24a[:, tl, 8:16], in_=cand_f)
                        nc.vector.match_replace(out=cand_f, in_to_replace=p24a[:, tl, 8:16],
                                                in_values=cand_f, imm_value=NEG)
                        mr = nc.vector.max(out=p24a[:, tl, 16:24], in_=cand_f)
                        R["cand"].read(mr)

                    def emit_fold(p24a):
                        # batched index recover: [128, NT, 20] strided view
                        pslice = p24a[:, :, 0:20]
                        r_f = wk.tile([128, NT, 20], F32, tag="r_f")
                        nc.vector.tensor_scalar(r_f[:], pslice, 2048.0, scalar2=None, op0=ALU.mult)
                        r32 = wk.tile([128, NT, 20], I32, tag="r32")
                        nc.vector.tensor_copy(out=r32[:], in_=r_f[:])
                        i32 = wk.tile([128, NT, 20], I32, tag="i32")
                        nc.vector.tensor_scalar(i32[:], r32[:], 2047, scalar2=None,
                                                op0=ALU.bitwise_and)
                        nc.vector.tensor_scalar(i32[:], i32[:], 1023, scalar2=None,
                                                op0=ALU.min)
                        # fold idx [128,NT,20] -> [128,NT,160] (f16 exact: idx <= 1023)
                        idxf = wk.tile([128, NT, 20], F16, tag="idxf")
                        cidx = nc.vector.tensor_copy(out=idxf[:], in_=i32[:])
                        R["idxf"].write(cidx)
                        # all 8 sel matmuls land in one PSUM tile; a single
                        # strided ACT copy emits the gather's (t, k, g8) index
                        # layout (the old per-g8 PE<->ACT ping-pong cost ~4us
                        # of span per round)
                        # each g8 slice starts at a 512B-aligned offset: a
                        # matmul output must not cross a PSUM bank boundary
                        fps = ps1.tile([128, 8, 128], F32, tag="fold")
                        idx16a = wk.tile([128, NT, 160], I16, tag="idx16a")
                        for g8 in range(8):
                            mm = nc.tensor.matmul(out=fps[:, g8, : NT * 20],
                                                  lhsT=sel_sb[:, g8, :],
                                                  rhs=idxf[:].rearrange("p t k -> p (t k)"),
                                                  start=True, stop=True, skip_group_check=True)
                            if g8 == 0:
                                R["fold"].write(mm)
                            if g8 == 7:
                                R["idxf"].read(mm)
                        fview = fps[:, :, : NT * 20].rearrange("p g (t k) -> p g t k", k=20)
                        cp = nc.scalar.copy(
                            out=idx16a[:].rearrange("p t (k g) -> p g t k", g=8),
                            in_=fview)
                        R["fold"].read(cp)
                        R["idx16a"].write(cp)
                        return idx16a

                    def stage_c(t, tl, idx16a):
                        # gather A^T rows; row i lands at [p, k, :]
                        g = gp.tile([128, K, GW], U32, tag=f"g{tl}")
                        gi = nc.gpsimd.dma_gather(
                            out_ap=g[:], in_ap=at_dram[:], idxs_ap=idx16a[:, tl, :],
                            num_idxs=K * 128, num_idxs_reg=K * 128, elem_size=GW,
                            single_packet=False)
                        R["idx16a"].read(gi)
                        R[f"g{tl}"].write(gi)
                        R["at_dram"].read(gi)

                        # max over k (DVE bf16 2x; pad cols beyond O ignored):
                        # 20->10->5->(4->2->1), then +slot4
                        gw = g[:].bitcast(BF16)
                        gv = gw if OP == O else gw[:, :, :O]
                        nc.vector.tensor_tensor(out=gv[:, 0:10, :], in0=gv[:, 0:10, :], in1=gv[:, 10:20, :], op=ALU.max)
                        nc.vector.tensor_tensor(out=gv[:, 0:5, :], in0=gv[:, 0:5, :], in1=gv[:, 5:10, :], op=ALU.max)
                        nc.vector.tensor_tensor(out=gv[:, 0:2, :], in0=gv[:, 0:2, :], in1=gv[:, 2:4, :], op=ALU.max)
                        mf = wk.tile([128, O], F32, tag="mf")
                        nc.vector.tensor_tensor(out=gv[:, 0, :], in0=gv[:, 0, :], in1=gv[:, 1, :], op=ALU.max)
                        last = nc.vector.tensor_tensor(out=mf[:], in0=gv[:, 0, :], in1=gv[:, 4, :], op=ALU.max)
                        R[f"g{tl}"].read(last)
                        R["mf"].write(last)

                        # Xnext[o, n] = Prelu(scale*(D + M^T) + bias)
                        for oc in range(OC):
                            ow = min(128, O - oc * 128)
                            xn_ps = psA.tile([128, 128], F32, tag="xn")
                            mmd = nc.tensor.matmul(out=xn_ps[:ow, :], lhsT=wd[:C, oc * 128 : oc * 128 + ow],
                                                   rhs=XS(ts(t, 128)),
                                                   start=True, stop=False)
                            R["xn"].write(mmd)
                            mmt = nc.tensor.matmul(out=xn_ps[:ow, :], lhsT=mf[:, oc * 128 : oc * 128 + ow],
                                                   rhs=ident[:], is_transpose=True,
                                                   start=False, stop=True, skip_group_check=True)
                            R["mf"].read(mmt)
                            if b == 0:
                                dst = x1_sb[:, ts(t, 128)]
                            elif b == 1:
                                dst = x2_sb[:, ts(t, 128)]
                            elif b == 2:
                                dst = x3_sb[:, ts(t, 128)]
                            else:
                                # x4 is only read (bf16) by the embedding --
                                # write it as bf16 directly
                                dst = x4_bf[:, oc, ts(t, 128)]
                            R["xn"].read(nc.scalar.activation(out=dst, in_=xn_ps[:ow, :], func=AF.Prelu,
                                                               bias=bi[:ow, oc : oc + 1], scale=sc[:ow, oc : oc + 1],
                                                               alpha=ALPHA))

                    if b == 2:
                        # x1/x2 are final once b2 starts; stage the embedding
                        # x12 input here, off b3's congested block start
                        nc.sync.dma_start(out=x12_sb[0:64, :], in_=x1_sb[:].bitcast(F32))
                        nc.sync.dma_start(out=x12_sb[64:128, :], in_=x2_sb[:].bitcast(F32))
                        nc.scalar.copy(out=x12_bf[:], in_=x12_sb[:])
                    if b == 3:
                        # embedding prep: runs during b3's A phase
                        nc.scalar.copy(out=x3_bf[:], in_=x3_sb[:].bitcast(F32))
                        esc, ebi = _bn_sb(bneT_sb)
                        hsums = wk1.tile([128, 8, 2], F32, tag="hsums")
                        pm01 = wk1.tile([128, 8, 2], F32, tag="pm01")
                        h_sb = wk1.tile([128, 16], F32, tag="h")
                        ebh = []
                        for jc in range(8):
                            ebf = wk1.tile([128, 2, 512], F32, tag=f"ebh{jc}", name=f"ebh{jc}")
                            ebh.append(ebf)

                    def embed_half(h):
                        # one jc-sweep of the 1024-ch embedding over column
                        # half h; emitted right after C(round h) so it overlaps
                        # the other round's gathers/trees
                        for jc in range(8):
                            pt = f"ppk{jc % 2}"
                            e_ps = psA.tile([128, 512], F32, tag=pt)
                            m0 = nc.tensor.matmul(out=e_ps[:], lhsT=weT_bf[:, 0, ts(jc, 128)],
                                             rhs=x12_bf[:, ts(h, 512)], start=True, stop=False)
                            R[pt].write(m0)
                            nc.tensor.matmul(out=e_ps[:], lhsT=weT_bf[:, 1, ts(jc, 128)],
                                             rhs=x3_bf[:, ts(h, 512)], start=False, stop=False)
                            nc.tensor.matmul(out=e_ps[:], lhsT=weT_bf[:, 2, ts(jc, 128)],
                                             rhs=x4_bf[:, 0, ts(h, 512)], start=False, stop=False)
                            nc.tensor.matmul(out=e_ps[:], lhsT=weT_bf[:, 3, ts(jc, 128)],
                                             rhs=x4_bf[:, 1, ts(h, 512)], start=False, stop=True)
                            # both halves land side by side in one tile; a
                            # single full-width reduce in the tail replaces
                            # two reduces + a combine and frees the C-phase
                            # DVE window
                            ac = nc.scalar.activation(out=ebh[jc][:, h, :], in_=e_ps[:], func=AF.Prelu,
                                                 bias=ebi[:, jc : jc + 1], scale=esc[:, jc : jc + 1],
                                                 alpha=ALPHA, accum_out=hsums[:, jc, h : h + 1])
                            R[pt].read(ac)
                            if h == 1:
                                nc.vector.tensor_reduce(out=pm01[:, jc, 1:2],
                                                        in_=ebh[jc][:, 1, :],
                                                        axis=AX.X, op=ALU.max)

                    # table feeds only the C phase; prioritized so it runs
                    # during the previous block's C instead of being starved
                    # behind the A-phase ACT work (b0: emitted after the first
                    # A tile instead -- there is no previous C to hide in, and
                    # at block start it would head-block the PE/ACT queues)
                    with tc.high_priority():
                        emit_at_table()
                    if b == 1:
                        # deferred big weight loads: SP queue is idle now and
                        # block 0's table stores are already queued ahead
                        nc.sync.dma_start(out=weT_bf[:], in_=weT[:].rearrange("(c p) n -> p c n", p=128))
                        nc.sync.dma_start(out=wf0T_sb[:], in_=wf0T[:].rearrange("(c p) n -> p c n", p=128))
                        nc.sync.dma_start(out=wf1T_sb[:], in_=wf1T[:].rearrange("(c p) n -> p c n", p=128))
                        nc.sync.dma_start(out=wfinT_sb[:], in_=wfinT[:].rearrange("(c p) n -> p c n", p=128))
                        nc.sync.dma_start(out=bfin_sb[:], in_=bfin[:])
                        nc.sync.dma_start(out=bneT_sb[:], in_=bneT[:].rearrange("(c p) q -> p c q", p=128))
                        nc.sync.dma_start(out=bnf0T_sb[:], in_=bnf0T[:].rearrange("(c p) q -> p c q", p=128))
                        nc.sync.dma_start(out=bnf1T_sb[:], in_=bnf1T[:].rearrange("(c p) q -> p c q", p=128))
                    # emit all A-phase work (topk + fold per round) before
                    # the C phases: C(r) has no DVE ops (except b3's first
                    # tree layer), so round r+1's topk overlaps round r's
                    # gathers/trees on Pool
                    idx_rounds = []
                    for r in range(T // NT):
                        p24a = wk.tile([128, NT, 24], F32, tag="p24a")
                        for tl in range(NT):
                            stage_a(r * NT + tl, tl, p24a)
                        with tc.high_priority():
                            idx_rounds.append(emit_fold(p24a))
                    for r in range(T // NT):
                        for tl in range(NT):
                            stage_c(r * NT + tl, tl, idx_rounds[r])
                        if b == 3:
                            if r == 1:
                                # the h0-half max reduces are ready now; they
                                # fill the DVE gap between the last tree and
                                # the first h1 activation
                                for jc in range(8):
                                    nc.vector.tensor_reduce(out=pm01[:, jc, 0:1],
                                                            in_=ebh[jc][:, 0, :],
                                                            axis=AX.X, op=ALU.max)
                            embed_half(r)

                # ---- pooling combine + head (inside the work pools) ----
                nc.vector.tensor_tensor(out=h_sb[:, 0:8], in0=pm01[:, :, 0], in1=pm01[:, :, 1], op=ALU.max)
                nc.vector.tensor_tensor(out=hsums[:, :, 0], in0=hsums[:, :, 0], in1=hsums[:, :, 1], op=ALU.add)
                nc.vector.tensor_scalar(h_sb[:, 8:16], hsums[:, :, 0], 1.0 / N,
                                        scalar2=None, op0=ALU.mult)
                if debug:
                    nc.sync.dma_start(out=dbg["x1d"][:], in_=x1_sb[:].bitcast(F32))
                    nc.sync.dma_start(out=dbg["x2d"][:], in_=x2_sb[:].bitcast(F32))
                    nc.sync.dma_start(out=dbg["x3d"][:], in_=x3_sb[:].bitcast(F32))
                    nc.sync.dma_start(out=dbg["hd"][:], in_=h_sb[:])

                f0sc, f0bi = _bn_sb(bnf0T_sb)
                h1_sb = wk1.tile([128, 4], F32, tag="h1")
                # independent per-oc PSUM banks (pd/ppk banks are idle by
                # now) so the four contraction chains don't serialize on WAR
                htags = ["pd0", "pd1", "ppk0", "ppk1"]
                for oc in range(4):
                    h1_ps = psA.tile([128, 1], F32, tag=htags[oc], name=f"h1_ps{oc}")
                    for kc in range(16):
                        mm = nc.tensor.matmul(out=h1_ps[:], lhsT=wf0T_sb[:, kc, ts(oc, 128)],
                                         rhs=h_sb[:, kc : kc + 1], start=(kc == 0), stop=(kc == 15))
                        if kc == 0:
                            R[htags[oc]].write(mm)
                    R[htags[oc]].read(nc.scalar.activation(out=h1_sb[:, oc : oc + 1], in_=h1_ps[:], func=AF.Prelu,
                                         bias=f0bi[:, oc : oc + 1], scale=f0sc[:, oc : oc + 1], alpha=ALPHA))
                f1sc, f1bi = _bn_sb(bnf1T_sb)
                h2_sb = wk1.tile([128, 2], F32, tag="h2")
                for oc in range(2):
                    h2_ps = psA.tile([128, 1], F32, tag=htags[oc], name=f"h2_ps{oc}")
                    for kc in range(4):
                        mm = nc.tensor.matmul(out=h2_ps[:], lhsT=wf1T_sb[:, kc, ts(oc, 128)],
                                         rhs=h1_sb[:, kc : kc + 1], start=(kc == 0), stop=(kc == 3))
                        if kc == 0:
                            R[htags[oc]].write(mm)
                    R[htags[oc]].read(nc.scalar.activation(out=h2_sb[:, oc : oc + 1], in_=h2_ps[:], func=AF.Prelu,
                                         bias=f1bi[:, oc : oc + 1], scale=f1sc[:, oc : oc + 1], alpha=ALPHA))
                o_ps = psA.tile([64, 1], F32, tag="xn")
                for kc in range(2):
                    mm = nc.tensor.matmul(out=o_ps[:], lhsT=wfinT_sb[:, kc, :],
                                     rhs=h2_sb[:, kc : kc + 1], start=(kc == 0), stop=(kc == 1))
                    if kc == 0:
                        R["xn"].write(mm)
                o_sb = wk1.tile([64, 1], F32, tag="osb")
                R["xn"].read(nc.scalar.activation(out=o_sb[:], in_=o_ps[:], func=AF.Prelu,
                                     bias=bfin_sb[:, 0:1], scale=1.0, alpha=1.0))
                nc.sync.dma_start(out=out_d[:], in_=o_sb[:])

    nc.compile()
    return nc


def _bn_prep(bn):
    """[4, O] (g, b, m, v) -> [O, 2] (scale, bias) in f32."""
    g, b, m, v = np.asarray(bn, np.float64)
    scale = g / np.sqrt(v + EPS)
    bias = b - m * scale
    return np.ascontiguousarray(np.stack([scale, bias], axis=1).astype(np.float32))


def _host_prep(inputs):
    """Per-core input maps: shard x over batch; weights transposed (layout only)."""
    f = np.ascontiguousarray
    selg = np.zeros((8, 128, 128), np.float16)
    for g in range(8):
        for r in range(128):
            selg[g, g * 16 + (r % 16), r] = 1.0
    common = {
        "weT": f(inputs["we"].T.astype(np.float32)).astype(_BF16),
        "bneT": _bn_prep(inputs["bne"]),
        "wf0T": f(inputs["wf0"].T.astype(np.float32)),
        "bnf0T": _bn_prep(inputs["bnf0"]),
        "wf1T": f(inputs["wf1"].T.astype(np.float32)),
        "bnf1T": _bn_prep(inputs["bnf1"]),
        "wfinT": f(inputs["wfin"].T.astype(np.float32)),
        "bfin": f(inputs["bfin"].astype(np.float32).reshape(64, 1)),
        "sel": selg,
        "onesd": np.ones((1, 128), np.float32),
        "monesd": np.full((128, 1), -0.5, np.float32),
        "iotaf": np.tile((np.arange(1024, dtype=np.float32) / 2048.0)
                         .astype(np.float16)[None, :], (128, 1)),
    }
    for i, (c, o) in enumerate(BLOCKS):
        w = np.asarray(inputs[f"w{i}"], np.float32)  # [O, 2C]
        common[f"wta{i}"] = f(w[:, :c].T)
        common[f"wtb{i}"] = f(w[:, c:].T)
        common[f"bnt{i}"] = _bn_prep(inputs[f"bn{i}"])
    xs = np.asarray(inputs["x"], np.float32)
    return [dict(common, x=f(xs[c]),
                 negxx0d=f((-0.5 * (xs[c].astype(np.float64) ** 2).sum(0))
                           .astype(np.float32).reshape(1, -1)))
            for c in range(xs.shape[0])]


_NC_CACHE = {}


def _get_nc(debug=False):
    if debug not in _NC_CACHE:
        _NC_CACHE[debug] = build(debug=debug)
    return _NC_CACHE[debug]


def run(inputs, debug=False, trace=False):
    nc = _get_nc(debug=debug)
    in_maps = _host_prep(inputs)
    res = run_bass_kernel_spmd(nc, in_maps, core_ids=list(range(len(in_maps))), trace=trace)
    out = np.stack([r["out"][:, 0] for r in res.results], axis=0)
    return out, res


def kernel(**inputs) -> np.ndarray:
    out, _ = run(inputs)
    return out.astype(np.float32)
